# revision 1
# baseline (speedup 1.0000x reference)
"""GCN forward on 8 Trainium2 NeuronCores.

Reference computation:
  h1 = relu(GCNConv(x, edge_index; w_conv, b_conv))      [20000, 32]
  h3 = relu(h1.flatten() @ w_fc1.T + b_fc1)              [128]
  out = relu(h3 @ w_fc2.T + b_fc2)                       [1, 20000]

Strategy (all 8 cores, SPMD, one NEFF):
  - GCNConv aggregation as a DENSE matmul: A_hat = D^-1/2 (A+I) D^-1/2 where
    (A+I) holds small integer edge counts, exactly representable in fp8e4m3
    (mixed fp8 lhsT x bf16 rhs matmul, fp32 psum accumulate -> no extra error).
    dinv[src] is folded into the H'' rows, dinv[dst] applied post-matmul.
    Indexed-gather alternatives measured 4-10x slower (SWDGE indirect DMA
    1.6us/128 rows; gpsimd ap_gather 26ns/idx), hence dense.
    Each core owns a 2500-node dst slice: psum[dst_tile] += Apack_tile.T @ H''.
  - H'' = dinv * (x @ w_conv) computed sharded (each core its 2500 src rows),
    then AllGather (bf16, 160KB/core).
  - fc1 column-sharded: core i dots its 80000 flat entries against its B
    slice, AllReduce of the [128] partials.
  - fc2 row-sharded: core i computes out[2500i:2500(i+1)].
"""
import numpy as np
import ml_dtypes

N = 20000
IN_FEAT = 128
CF = 32            # conv out feats
FC1 = 128
NC_ = 8            # cores
NS = N // NC_      # 2500 nodes per core
DT = 20            # dst tiles per core (last partial: 68 rows)
KT = (N + 127) // 128  # 157 src tiles
KT_A = 79          # first  src-tile chunk
KT_B = KT - KT_A   # second src-tile chunk (78)

_BF16 = ml_dtypes.bfloat16
_F8 = ml_dtypes.float8_e4m3


def _host_prep(x, edge_index, w_conv, b_conv, w_fc1, b_fc1, w_fc2, b_fc2):
    src = edge_index[0].astype(np.int64)
    dst = edge_index[1].astype(np.int64)
    deg = np.bincount(dst, minlength=N).astype(np.float32) + 1.0
    dinv = (1.0 / np.sqrt(deg)).astype(np.float32)

    x = np.asarray(x, np.float32)
    w_conv = np.asarray(w_conv, np.float32)
    b_conv = np.asarray(b_conv, np.float32)
    w_fc1 = np.asarray(w_fc1, np.float32)
    b_fc1 = np.asarray(b_fc1, np.float32)
    w_fc2 = np.asarray(w_fc2, np.float32)
    b_fc2 = np.asarray(b_fc2, np.float32)

    lut = np.arange(16).astype(_F8)  # exact small-int -> fp8e4m3
    bconvb = np.ascontiguousarray(np.broadcast_to(b_conv[None, :], (128, CF)))
    bfc1c = np.ascontiguousarray(b_fc1.reshape(128, 1))

    in_maps = []
    for c in range(NC_):
        base = c * NS
        # xt: [128 feat, 2560 nodes] zero-padded
        xt = np.zeros((IN_FEAT, DT * 128), np.float32)
        xt[:, :NS] = x[base:base + NS].T
        # dinv tile [128, 20], zero-padded
        dv = np.zeros(DT * 128, np.float32)
        dv[:NS] = dinv[base:base + NS]
        dv = np.ascontiguousarray(dv.reshape(DT, 128).T)
        # A_pack[d, p, k*128+j] = count(src=128k+p -> dst=base+128d+j) + selfloop
        cnt = np.zeros((DT, 128, KT * 128), np.uint8)
        m = (dst >= base) & (dst < base + NS)
        s, dl = src[m], dst[m] - base
        np.add.at(cnt, (dl // 128, s % 128, (s // 128) * 128 + dl % 128), 1)
        v = np.arange(base, base + NS)
        np.add.at(cnt, ((v - base) // 128, v % 128, (v // 128) * 128 + (v - base) % 128), 1)
        assert cnt.max() < 16, cnt.max()
        apack = lut[cnt]
        del cnt
        # B_pack[k, n, c, o] = w_fc1[o, 80000*i + 32*(128k+n) + c], zero-padded
        w1 = w_fc1[:, base * CF:(base + NS) * CF]  # [128, 80000]
        bp = np.zeros((DT, 128, CF, FC1), _BF16)
        bp[:19] = w1[:, :19 * 128 * CF].reshape(FC1, 19, 128, CF).transpose(1, 2, 3, 0).astype(_BF16)
        bp[19, :NS - 19 * 128] = w1[:, 19 * 128 * CF:].reshape(FC1, NS - 19 * 128, CF).transpose(1, 2, 0).astype(_BF16)
        in_maps.append({
            "xt": xt,
            "wconv": np.ascontiguousarray(w_conv),
            "dinv": dv,
            "bconvb": bconvb,
            "apack": apack.reshape(DT, 128, KT * 128),
            "bpack": np.ascontiguousarray(bp.reshape(DT, 128, CF * FC1)),
            "bfc1": bfc1c,
            "w2t": np.ascontiguousarray(w_fc2[base:base + NS].T),
            "bfc2": np.ascontiguousarray(b_fc2[base:base + NS].reshape(1, NS)),
        })
    return in_maps


def _build_bass(timing_reps=None):
    import concourse.bass as bass
    import concourse.mybir as mybir
    import concourse.tile as tile
    from concourse import bacc

    F32, BF16, F8 = mybir.dt.float32, mybir.dt.bfloat16, mybir.dt.float8e4
    import contextlib
    nc = bacc.Bacc("TRN2", target_bir_lowering=False, debug=False,
                   num_devices=1 if timing_reps else NC_)

    xt = nc.dram_tensor("xt", [IN_FEAT, DT * 128], F32, kind="ExternalInput")
    wconv = nc.dram_tensor("wconv", [IN_FEAT, CF], F32, kind="ExternalInput")
    dinv = nc.dram_tensor("dinv", [128, DT], F32, kind="ExternalInput")
    bconvb = nc.dram_tensor("bconvb", [128, CF], F32, kind="ExternalInput")
    apack = nc.dram_tensor("apack", [DT, 128, KT * 128], F8, kind="ExternalInput")
    bpack = nc.dram_tensor("bpack", [DT, 128, CF * FC1], BF16, kind="ExternalInput")
    bfc1 = nc.dram_tensor("bfc1", [FC1, 1], F32, kind="ExternalInput")
    w2t = nc.dram_tensor("w2t", [FC1, NS], F32, kind="ExternalInput")
    bfc2 = nc.dram_tensor("bfc2", [1, NS], F32, kind="ExternalInput")
    out = nc.dram_tensor("out", [1, NS], F32, kind="ExternalOutput")

    hq_in = nc.dram_tensor("hq_in", [NS, CF], BF16)
    hq_out = nc.dram_tensor("hq_out", [N, CF], BF16, addr_space="Shared")
    p_in = nc.dram_tensor("p_in", [FC1, 1], F32)
    p_out = nc.dram_tensor("p_out", [FC1, 1], F32, addr_space="Shared")

    with tile.TileContext(nc) as tc:
        with tc.tile_pool(name="const", bufs=1) as cp, \
             tc.tile_pool(name="work", bufs=4) as wp, \
             tc.tile_pool(name="ps", bufs=2, space="PSUM") as pp, \
             tc.tile_pool(name="ps1", bufs=1, space="PSUM") as pp1:

            xt_sb = cp.tile([IN_FEAT, DT * 128], F32, tag="xt")
            nc.sync.dma_start(out=xt_sb[:], in_=xt[:])
            wconv_sb = cp.tile([IN_FEAT, CF], F32, tag="wconv")
            nc.sync.dma_start(out=wconv_sb[:], in_=wconv[:])
            dinv_sb = cp.tile([128, DT], F32, tag="dinv")
            nc.sync.dma_start(out=dinv_sb[:], in_=dinv[:])
            bconvb_sb = cp.tile([128, CF], F32, tag="bconvb")
            nc.sync.dma_start(out=bconvb_sb[:], in_=bconvb[:])
            bfc1_sb = cp.tile([FC1, 1], F32, tag="bfc1")
            nc.sync.dma_start(out=bfc1_sb[:], in_=bfc1[:])
            w2t_sb = cp.tile([FC1, NS], F32, tag="w2t")
            nc.sync.dma_start(out=w2t_sb[:], in_=w2t[:])
            bfc2_sb = cp.tile([1, NS], F32, tag="bfc2")
            nc.sync.dma_start(out=bfc2_sb[:], in_=bfc2[:])

            loop_cm = tc.For_i(0, timing_reps, 1) if timing_reps else contextlib.nullcontext()
            loop_cm.__enter__()

            # ---- S1: H'' = dinv * (x @ w_conv) for own src slice (bf16) ----
            hq_sb = cp.tile([128, DT * CF], BF16, tag="hq")
            for k in range(DT):
                ps = pp.tile([128, CF], F32, space="PSUM", tag="ps")
                nc.tensor.matmul(out=ps[:], lhsT=xt_sb[:, k * 128:(k + 1) * 128],
                                 rhs=wconv_sb[:], start=True, stop=True)
                nc.vector.tensor_tensor(out=hq_sb[:, k * CF:(k + 1) * CF], in0=ps[:],
                                        in1=dinv_sb[:, k:k + 1].to_broadcast([128, CF]),
                                        op=mybir.AluOpType.mult)
            # store rows 0:2432 then tail 2432:2500
            nc.sync.dma_start(out=hq_in[:19 * 128].rearrange("(k p) f -> p k f", p=128),
                              in_=hq_sb[:, :19 * CF].rearrange("p (k f) -> p k f", f=CF))
            nc.sync.dma_start(out=hq_in[19 * 128:NS], in_=hq_sb[:NS - 19 * 128, 19 * CF:20 * CF])

            # ---- S2: AllGather H'' ----
            if timing_reps:
                nc.sync.dma_start(out=hq_out[:NS], in_=hq_in[:])
            else:
                nc.gpsimd.collective_compute(
                    "AllGather", mybir.AluOpType.bypass,
                    replica_groups=[list(range(NC_))],
                    ins=[hq_in[:]], outs=[hq_out[:]])

            # ---- S3: load full H'' into SBUF [128, 157*32] ----
            hp = cp.tile([128, KT * CF], BF16, tag="hp")
            nc.vector.memset(hp[:], 0.0)
            nc.sync.dma_start(out=hp[:, :156 * CF].rearrange("p (k f) -> p k f", f=CF),
                              in_=hq_out[:156 * 128].rearrange("(k p) f -> p k f", p=128))
            nc.sync.dma_start(out=hp[:N - 156 * 128, 156 * CF:], in_=hq_out[156 * 128:])

            # ---- S4: aggregation, one dst tile at a time ----
            h1_sb = cp.tile([128, DT * CF], BF16, tag="h1")
            for d in range(DT):
                a1 = wp.tile([128, KT * 128], F8, tag="apk", bufs=3)
                nc.sync.dma_start(out=a1[:], in_=apack[d])
                psd = pp.tile([128, CF], F32, space="PSUM", tag="ps")
                for k in range(KT):
                    nc.tensor.matmul(out=psd[:], lhsT=a1[:, k * 128:(k + 1) * 128],
                                     rhs=hp[:, k * CF:(k + 1) * CF],
                                     start=(k == 0), stop=(k == KT - 1))
                t = wp.tile([128, CF], F32, tag="ep")
                nc.vector.tensor_tensor(out=t[:], in0=psd[:],
                                        in1=dinv_sb[:, d:d + 1].to_broadcast([128, CF]),
                                        op=mybir.AluOpType.mult)
                nc.vector.tensor_tensor(out=t[:], in0=t[:], in1=bconvb_sb[:],
                                        op=mybir.AluOpType.add)
                nc.scalar.activation(out=h1_sb[:, d * CF:(d + 1) * CF], in_=t[:],
                                     func=mybir.ActivationFunctionType.Relu)

            # ---- S5: fc1 partial: ps1[128, 1] += B_tile.T @ h1_col ----
            ps1 = pp1.tile([FC1, 1], F32, space="PSUM", tag="ps1")
            for k in range(DT):
                bsb = wp.tile([128, CF * FC1], BF16, tag="bpk")
                nc.sync.dma_start(out=bsb[:], in_=bpack[k])
                for c in range(CF):
                    nc.tensor.matmul(out=ps1[:], lhsT=bsb[:, c * FC1:(c + 1) * FC1],
                                     rhs=h1_sb[:, k * CF + c:k * CF + c + 1],
                                     start=(k == 0 and c == 0), stop=(k == DT - 1 and c == CF - 1))
            p_sb = cp.tile([FC1, 1], F32, tag="p_sb")
            nc.vector.tensor_copy(out=p_sb[:], in_=ps1[:])
            nc.gpsimd.dma_start(out=p_in[:], in_=p_sb[:])

            # ---- S6: AllReduce partials, h3 = relu(sum + b_fc1) ----
            if timing_reps:
                nc.sync.dma_start(out=p_out[:], in_=p_in[:])
            else:
                nc.gpsimd.collective_compute(
                    "AllReduce", mybir.AluOpType.add,
                    replica_groups=[list(range(NC_))],
                    ins=[p_in[:]], outs=[p_out[:]])
            h3 = cp.tile([FC1, 1], F32, tag="h3")
            nc.sync.dma_start(out=h3[:], in_=p_out[:])
            nc.vector.tensor_tensor(out=h3[:], in0=h3[:], in1=bfc1_sb[:],
                                    op=mybir.AluOpType.add)
            nc.scalar.activation(out=h3[:], in_=h3[:],
                                 func=mybir.ActivationFunctionType.Relu)

            # ---- S7: fc2 slice: out = relu(h3.T @ w2t + bfc2) ----
            o_sb = cp.tile([1, NS], F32, tag="o_sb")
            for j in range(5):
                ps2 = pp.tile([1, 500], F32, space="PSUM", tag="ps2")
                nc.tensor.matmul(out=ps2[:], lhsT=h3[:],
                                 rhs=w2t_sb[:, j * 500:(j + 1) * 500], start=True, stop=True)
                nc.vector.tensor_tensor(out=o_sb[:, j * 500:(j + 1) * 500], in0=ps2[:],
                                        in1=bfc2_sb[:, j * 500:(j + 1) * 500],
                                        op=mybir.AluOpType.add)
            nc.scalar.activation(out=o_sb[:], in_=o_sb[:],
                                 func=mybir.ActivationFunctionType.Relu)
            nc.sync.dma_start(out=out[:], in_=o_sb[:])
            loop_cm.__exit__(None, None, None) if timing_reps else None

    nc.finalize()
    return nc


_CACHED = {}


def kernel(**inputs) -> np.ndarray:
    from concourse.bass_utils import run_bass_kernel_spmd

    in_maps = _host_prep(**inputs)
    if "nc" not in _CACHED:
        _CACHED["nc"] = _build_bass()
    nc = _CACHED["nc"]
    res = run_bass_kernel_spmd(nc, in_maps, core_ids=list(range(NC_)))
    return np.concatenate([res.results[c]["out"] for c in range(NC_)], axis=1)



# revision 8
# speedup vs baseline: 1.3347x; 1.3347x over previous
"""GCN forward on 8 Trainium2 NeuronCores.

Reference computation:
  h1 = relu(GCNConv(x, edge_index; w_conv, b_conv))      [20000, 32]
  h3 = relu(h1.flatten() @ w_fc1.T + b_fc1)              [128]
  out = relu(h3 @ w_fc2.T + b_fc2)                       [1, 20000]

Strategy (all 8 cores, SPMD, one NEFF):
  - GCNConv aggregation as a DENSE matmul with the A-matrix as the MOVING
    operand: A_hat = D^-1/2 (A+I) D^-1/2 where (A+I) holds small integer
    edge counts, exact in fp8e4m3. dinv[src] folds into H'' rows, dinv[dst]
    applied post-matmul. Stationary = H'' k-tile [128 src, 32 feat] (cheap
    32-col weight loads), moving = A k-tile [128 src, 500 dst] fp8 ->
    PE streams A at 1 col/cycle; 5 persistent PSUM banks [32, 500]
    accumulate over all 157 src tiles. (The old orientation - A as
    stationary - paid a 128-col LDWEIGHTS per 32-col matmul: ~2x slower.)
  - A packed per-partition-contiguous [128, 157*2500] fp8, streamed in
    8-k-tile (2.5 MB) chunks, triple buffered.
  - H'' = dinv * (x @ w_conv) computed sharded, AllGather (bf16).
  - fc1 column-sharded: h1 [32, 2500] -> PE-transpose -> h1T [128, 640];
    640 matmuls lhsT = w1-chunk [128, 128] bf16 (FWL), rhs = h1T column,
    accumulating [128, 1]; w1 bf16 (21 MB/core) streamed in 64-col chunks,
    prefetched during aggregation. AllReduce the [128, 1] partials.
  - fc2 column-blocked: 20 matmuls lhsT = w2-block [128, 128] bf16,
    rhs = h3 [128, 1] -> psum [128, 20]; 128-lane epilogue; host
    un-permutes the [128, 20] block layout.
"""
import numpy as np
import ml_dtypes

N = 20000
IN_FEAT = 128
CF = 32            # conv out feats
FC1 = 128
NC_ = 8            # cores
NS = N // NC_      # 2500 nodes per core
DT = 20            # own-slice node tiles (last partial: 68 rows)
KT = (N + 127) // 128  # 157 src tiles (last partial: 32 rows)
JW = 5             # dst chunks of 500
JC = NS // JW      # 500 dst cols per chunk
CK = 8             # A k-tiles per DMA chunk (2.5 MB)
NCH = (KT + CK - 1) // CK  # 20 A chunks (last: 5)
GK = 64            # fc1 g-columns per DMA chunk (2.1 MB)
NG = 640           # fc1 contraction chunks (20 blocks * 32 feats)
W1BUFS = 4         # fc1 weight chunk buffers (prefetch depth)

_BF16 = ml_dtypes.bfloat16
_F8 = ml_dtypes.float8_e4m3


def _host_prep(x, edge_index, w_conv, b_conv, w_fc1, b_fc1, w_fc2, b_fc2):
    src = edge_index[0].astype(np.int64)
    dst = edge_index[1].astype(np.int64)
    deg = np.bincount(dst, minlength=N).astype(np.float32) + 1.0
    dinv = (1.0 / np.sqrt(deg)).astype(np.float32)

    x = np.asarray(x, np.float32)
    w_conv = np.asarray(w_conv, np.float32)
    b_conv = np.asarray(b_conv, np.float32)
    w_fc1 = np.asarray(w_fc1, np.float32)
    b_fc1 = np.asarray(b_fc1, np.float32)
    w_fc2 = np.asarray(w_fc2, np.float32)
    b_fc2 = np.asarray(b_fc2, np.float32)

    lut = np.arange(16).astype(_F8)  # exact small-int -> fp8e4m3
    ident = np.eye(CF, dtype=np.float32)
    wconvb = w_conv.astype(_BF16)
    bconvc = np.ascontiguousarray(b_conv.reshape(CF, 1))
    bfc1c = np.ascontiguousarray(b_fc1.reshape(FC1, 1))

    in_maps = []
    for c in range(NC_):
        base = c * NS
        # xt: [128 feat, 2560 nodes] bf16 zero-padded
        xt = np.zeros((IN_FEAT, DT * 128), _BF16)
        xt[:, :NS] = x[base:base + NS].T.astype(_BF16)
        # dinv own-slice tile layout [128, 20], zero-padded
        dv = np.zeros(DT * 128, np.float32)
        dv[:NS] = dinv[base:base + NS]
        dv = np.ascontiguousarray(dv.reshape(DT, 128).T)
        # dinv dst replicated across 32 feat partitions [32, 2500]
        dvrep = np.ascontiguousarray(
            np.broadcast_to(dinv[base:base + NS][None, :], (CF, NS)))
        # A_pack[p, k*2500 + j] = count(src=128k+p -> dst=base+j) + selfloop
        cnt = np.zeros((128, KT * NS), np.uint8)
        m = (dst >= base) & (dst < base + NS)
        s, dl = src[m], dst[m] - base
        np.add.at(cnt, (s % 128, (s // 128) * NS + dl), 1)
        v = np.arange(base, base + NS)
        np.add.at(cnt, (v % 128, (v // 128) * NS + (v - base)), 1)
        assert cnt.max() < 16, cnt.max()
        apack = lut[cnt]
        del cnt
        # w1pack[p, g*128 + o] = w_fc1[o, 32*(base+128b+p) + f], g = 32b+f
        w1c = w_fc1[:, base * CF:(base + NS) * CF]          # [128, 80000]
        wpad = np.zeros((FC1, DT * 128, CF), np.float32)    # [o, nl, f]
        wpad[:, :NS] = w1c.reshape(FC1, NS, CF)
        # -> [p, b, f, o] -> [128, 640*128]
        w1p = wpad.reshape(FC1, DT, 128, CF).transpose(2, 1, 3, 0)
        w1p = np.ascontiguousarray(w1p.reshape(128, NG * FC1)).astype(_BF16)
        # w2pack[p, b*128 + q] = w_fc2[base + 128b + q, p]
        w2pad = np.zeros((DT * 128, FC1), np.float32)
        w2pad[:NS] = w_fc2[base:base + NS]
        w2p = w2pad.reshape(DT, 128, FC1).transpose(2, 0, 1)
        w2p = np.ascontiguousarray(w2p.reshape(FC1, DT * 128)).astype(_BF16)
        # bias2[q, b] = b_fc2[base + 128b + q] (zero pad)
        b2 = np.zeros(DT * 128, np.float32)
        b2[:NS] = b_fc2[base:base + NS]
        b2 = np.ascontiguousarray(b2.reshape(DT, 128).T)
        in_maps.append({
            "xt": xt,
            "wconv": wconvb,
            "dinv20": dv,
            "dvrep": dvrep,
            "bconv": bconvc,
            "ident": ident,
            "apack": apack,
            "w1pack": w1p,
            "bfc1": bfc1c,
            "w2pack": w2p,
            "bias2": b2,
        })
    return in_maps


def _build_bass(timing_reps=None):
    import concourse.bass as bass
    import concourse.mybir as mybir
    import concourse.tile as tile
    from concourse import bacc

    F32, BF16, F8 = mybir.dt.float32, mybir.dt.bfloat16, mybir.dt.float8e4
    import contextlib
    nc = bacc.Bacc("TRN2", target_bir_lowering=False, debug=False,
                   num_devices=1 if timing_reps else NC_)

    xt = nc.dram_tensor("xt", [IN_FEAT, DT * 128], BF16, kind="ExternalInput")
    wconv = nc.dram_tensor("wconv", [IN_FEAT, CF], BF16, kind="ExternalInput")
    dinv20 = nc.dram_tensor("dinv20", [128, DT], F32, kind="ExternalInput")
    dvrep = nc.dram_tensor("dvrep", [CF, NS], F32, kind="ExternalInput")
    bconv = nc.dram_tensor("bconv", [CF, 1], F32, kind="ExternalInput")
    ident = nc.dram_tensor("ident", [CF, CF], F32, kind="ExternalInput")
    apack = nc.dram_tensor("apack", [128, KT * NS], F8, kind="ExternalInput")
    w1pack = nc.dram_tensor("w1pack", [128, NG * FC1], BF16, kind="ExternalInput")
    bfc1 = nc.dram_tensor("bfc1", [FC1, 1], F32, kind="ExternalInput")
    w2pack = nc.dram_tensor("w2pack", [FC1, DT * 128], BF16, kind="ExternalInput")
    bias2 = nc.dram_tensor("bias2", [128, DT], F32, kind="ExternalInput")
    out = nc.dram_tensor("out", [128, DT], F32, kind="ExternalOutput")

    hq_in = nc.dram_tensor("hq_in", [NS, CF], BF16)
    hq_out = nc.dram_tensor("hq_out", [N, CF], BF16, addr_space="Shared")
    p_in = nc.dram_tensor("p_in", [FC1, 1], F32)
    p_out = nc.dram_tensor("p_out", [FC1, 1], F32, addr_space="Shared")

    with tile.TileContext(nc) as tc:
        with tc.tile_pool(name="const", bufs=1) as cp, \
             tc.tile_pool(name="abuf", bufs=3) as apool, \
             tc.tile_pool(name="w1buf", bufs=W1BUFS) as wpool, \
             tc.tile_pool(name="work", bufs=2) as wp, \
             tc.tile_pool(name="psA", bufs=2, space="PSUM") as ppA, \
             tc.tile_pool(name="psB", bufs=1, space="PSUM") as ppB, \
             tc.tile_pool(name="psC", bufs=1, space="PSUM") as ppC:

            xt_sb = cp.tile([IN_FEAT, DT * 128], BF16, tag="xt")
            nc.sync.dma_start(out=xt_sb[:], in_=xt[:])
            wconv_sb = cp.tile([IN_FEAT, CF], BF16, tag="wconv")
            nc.sync.dma_start(out=wconv_sb[:], in_=wconv[:])
            dinv20_sb = cp.tile([128, DT], F32, tag="dinv20")
            nc.sync.dma_start(out=dinv20_sb[:], in_=dinv20[:])
            dvrep_sb = cp.tile([CF, NS], F32, tag="dvrep")
            nc.sync.dma_start(out=dvrep_sb[:], in_=dvrep[:])
            bconv_sb = cp.tile([CF, 1], F32, tag="bconv")
            nc.sync.dma_start(out=bconv_sb[:], in_=bconv[:])
            ident_sb = cp.tile([CF, CF], F32, tag="ident")
            nc.sync.dma_start(out=ident_sb[:], in_=ident[:])
            bfc1_sb = cp.tile([FC1, 1], F32, tag="bfc1")
            nc.sync.dma_start(out=bfc1_sb[:], in_=bfc1[:])
            w2_sb = cp.tile([FC1, DT * 128], BF16, tag="w2")
            nc.sync.dma_start(out=w2_sb[:], in_=w2pack[:])
            bias2_sb = cp.tile([128, DT], F32, tag="bias2")
            nc.sync.dma_start(out=bias2_sb[:], in_=bias2[:])

            # hp pad columns (src tile 156, partitions 32:128) stay zero;
            # the per-iteration DMAs only overwrite real rows, so memset once.
            hp = cp.tile([128, KT * CF], BF16, tag="hp")
            nc.vector.memset(hp[:], 0.0)

            loop_cm = tc.For_i(0, timing_reps, 1) if timing_reps else contextlib.nullcontext()
            loop_cm.__enter__()

            # ---- S1: H'' = dinv * (x @ w_conv) for own src slice (bf16) ----
            hq_sb = cp.tile([128, DT * CF], BF16, tag="hq")
            for k in range(DT):
                ps = ppA.tile([128, CF], F32, space="PSUM", tag="mm")
                nc.tensor.matmul(out=ps[:], lhsT=xt_sb[:, k * 128:(k + 1) * 128],
                                 rhs=wconv_sb[:], start=True, stop=True)
                nc.vector.tensor_tensor(out=hq_sb[:, k * CF:(k + 1) * CF], in0=ps[:],
                                        in1=dinv20_sb[:, k:k + 1].to_broadcast([128, CF]),
                                        op=mybir.AluOpType.mult)
            # store rows 0:2432 then tail 2432:2500
            nc.sync.dma_start(out=hq_in[:19 * 128].rearrange("(k p) f -> p k f", p=128),
                              in_=hq_sb[:, :19 * CF].rearrange("p (k f) -> p k f", f=CF))
            nc.sync.dma_start(out=hq_in[19 * 128:NS], in_=hq_sb[:NS - 19 * 128, 19 * CF:20 * CF])

            # ---- S2: AllGather H'' ----
            if timing_reps:
                nc.sync.dma_start(out=hq_out[:NS], in_=hq_in[:])
            else:
                nc.gpsimd.collective_compute(
                    "AllGather", mybir.AluOpType.bypass,
                    replica_groups=[list(range(NC_))],
                    ins=[hq_in[:]], outs=[hq_out[:]])

            # ---- prefetch: first 3 A chunks (free bufs, no deps) run during
            # the AllGather; emit them BEFORE the hp load so they don't queue
            # behind its AG-gated DMA.
            def a_chunk_dma(t):
                k0 = t * CK
                ck = min(CK, KT - k0)
                ab = apool.tile([128, CK * NS], F8, tag="apk")
                nc.sync.dma_start(out=ab[:, :ck * NS],
                                  in_=apack[:, k0 * NS:(k0 + ck) * NS])
                return ab

            def w1_chunk_dma(t):
                wb = wpool.tile([128, GK * FC1], BF16, tag="w1")
                nc.sync.dma_start(out=wb[:],
                                  in_=w1pack[:, t * GK * FC1:(t + 1) * GK * FC1])
                return wb

            abufs = {t: a_chunk_dma(t) for t in range(3)}

            # ---- S3: load full H'' into SBUF [128, 157*32] ----
            nc.sync.dma_start(out=hp[:, :156 * CF].rearrange("p (k f) -> p k f", f=CF),
                              in_=hq_out[:156 * 128].rearrange("(k p) f -> p k f", p=128))
            nc.sync.dma_start(out=hp[:N - 156 * 128, 156 * CF:], in_=hq_out[156 * 128:])

            # ---- S4: aggregation: psum[32, 500] x5 += hp_k.T @ A_chunk ----
            # w1 chunks 0..2 prefetch mid-agg (W1BUFS-1 free buffers -> no
            # queue stall); remaining chunks stream during S7.
            wbufs = {}
            psj = [ppB.tile([CF, JC], F32, space="PSUM", tag=f"agg{j}",
                            name=f"agg{j}")
                   for j in range(JW)]
            for t in range(NCH):
                k0 = t * CK
                ck = min(CK, KT - k0)
                ab = abufs.pop(t) if t in abufs else a_chunk_dma(t)
                if t in (4, 8, 12, 16):
                    wt = (4, 8, 12, 16).index(t)
                    wbufs[wt] = w1_chunk_dma(wt)
                for kl in range(ck):
                    k = k0 + kl
                    for j in range(JW):
                        nc.tensor.matmul(
                            out=psj[j][:],
                            lhsT=hp[:, k * CF:(k + 1) * CF],
                            rhs=ab[:, kl * NS + j * JC:kl * NS + (j + 1) * JC],
                            start=(k == 0), stop=(k == KT - 1))

            # ---- S5: h1 = relu(dinv_dst * agg + b_conv), f32 [32, 2500] ----
            h1 = cp.tile([CF, NS], F32, tag="h1")
            for j in range(JW):
                t1 = wp.tile([CF, JC], F32, tag="ep")
                nc.vector.tensor_tensor(out=t1[:], in0=psj[j][:],
                                        in1=dvrep_sb[:, j * JC:(j + 1) * JC],
                                        op=mybir.AluOpType.mult)
                nc.vector.tensor_tensor(out=t1[:], in0=t1[:],
                                        in1=bconv_sb[:].to_broadcast([CF, JC]),
                                        op=mybir.AluOpType.add)
                nc.scalar.activation(out=h1[:, j * JC:(j + 1) * JC], in_=t1[:],
                                     func=mybir.ActivationFunctionType.Relu)

            # ---- S6: h1T [128, 640] bf16 via PE transpose ----
            h1T = cp.tile([128, DT * CF], BF16, tag="h1T")
            for b in range(DT):
                n0 = b * 128
                nw = min(128, NS - n0)
                tp = ppA.tile([128, CF], F32, space="PSUM", tag="mm")
                if nw < 128:
                    nc.vector.memset(tp[:], 0.0)
                nc.tensor.transpose(out=tp[:nw], in_=h1[:, n0:n0 + nw],
                                    identity=ident_sb[:])
                nc.vector.tensor_copy(out=h1T[:, b * CF:(b + 1) * CF], in_=tp[:])

            # ---- S7: fc1 partial: ps[128, 1] += w1chunk.T @ h1T_col ----
            psf = ppC.tile([128, 24], F32, space="PSUM", tag="fc")
            for t in range(NG // GK):
                wb = wbufs.pop(t) if t in wbufs else w1_chunk_dma(t)
                for gl in range(GK):
                    g = t * GK + gl
                    nc.tensor.matmul(out=psf[:, 0:1],
                                     lhsT=wb[:, gl * FC1:(gl + 1) * FC1],
                                     rhs=h1T[:, g:g + 1],
                                     start=(g == 0), stop=(g == NG - 1))
                # lookahead: buffer (t % W1BUFS) is free once chunk t's
                # matmuls are emitted; stream chunk t+W1BUFS behind them
                if t + W1BUFS < NG // GK:
                    wbufs[t + W1BUFS] = w1_chunk_dma(t + W1BUFS)
            p_sb = cp.tile([FC1, 1], F32, tag="p_sb")
            nc.vector.tensor_copy(out=p_sb[:], in_=psf[:, 0:1])
            nc.gpsimd.dma_start(out=p_in[:], in_=p_sb[:])

            # ---- S8: AllReduce partials, h3 = relu(sum + b_fc1) bf16 ----
            if timing_reps:
                nc.sync.dma_start(out=p_out[:], in_=p_in[:])
            else:
                nc.gpsimd.collective_compute(
                    "AllReduce", mybir.AluOpType.add,
                    replica_groups=[list(range(NC_))],
                    ins=[p_in[:]], outs=[p_out[:]])
            h3f = cp.tile([FC1, 1], F32, tag="h3f")
            nc.sync.dma_start(out=h3f[:], in_=p_out[:])
            nc.vector.tensor_tensor(out=h3f[:], in0=h3f[:], in1=bfc1_sb[:],
                                    op=mybir.AluOpType.add)
            h3 = cp.tile([FC1, 1], BF16, tag="h3")
            nc.scalar.activation(out=h3[:], in_=h3f[:],
                                 func=mybir.ActivationFunctionType.Relu)

            # ---- S9: fc2 blocks: psum[128, 20]; epilogue on 128 lanes ----
            for b in range(DT):
                nc.tensor.matmul(out=psf[:, 4 + b:5 + b],
                                 lhsT=w2_sb[:, b * 128:(b + 1) * 128],
                                 rhs=h3[:], start=(b == 0), stop=(b == DT - 1))
            o_sb = cp.tile([128, DT], F32, tag="o_sb")
            nc.vector.tensor_tensor(out=o_sb[:], in0=psf[:, 4:4 + DT],
                                    in1=bias2_sb[:], op=mybir.AluOpType.add)
            nc.scalar.activation(out=o_sb[:], in_=o_sb[:],
                                 func=mybir.ActivationFunctionType.Relu)
            nc.sync.dma_start(out=out[:], in_=o_sb[:])
            loop_cm.__exit__(None, None, None) if timing_reps else None

    nc.finalize()
    return nc


_CACHED = {}


def kernel(**inputs) -> np.ndarray:
    from concourse.bass_utils import run_bass_kernel_spmd

    in_maps = _host_prep(**inputs)
    if "nc" not in _CACHED:
        _CACHED["nc"] = _build_bass()
    nc = _CACHED["nc"]
    res = run_bass_kernel_spmd(nc, in_maps, core_ids=list(range(NC_)))
    # out[p, b] = q-value for node base + 128*b + p
    outs = []
    for c in range(NC_):
        o = np.asarray(res.results[c]["out"])  # [128, 20]
        outs.append(o.T.reshape(-1)[:NS])
    return np.concatenate(outs).reshape(1, N)


# revision 12
# speedup vs baseline: 1.6578x; 1.2421x over previous
"""GCN forward on 8 Trainium2 NeuronCores.

Reference computation:
  h1 = relu(GCNConv(x, edge_index; w_conv, b_conv))      [20000, 32]
  h3 = relu(h1.flatten() @ w_fc1.T + b_fc1)              [128]
  out = relu(h3 @ w_fc2.T + b_fc2)                       [1, 20000]

Strategy (all 8 cores, SPMD, one NEFF):
  - GCNConv aggregation as a DENSE matmul with the A-matrix as the MOVING
    operand: A_hat = D^-1/2 (A+I) D^-1/2; (A+I) holds small integer edge
    counts, exact in fp8e4m3. dinv[src] folds into x rows on host,
    dinv[dst] applied post-matmul. Stationary = H'' tile [128 src, 32 feat]
    (cheap 32-col weight loads), moving = A tile [128 src, 500 dst] fp8 ->
    PE streams A at 1 col/cycle; 5 persistent PSUM banks [32, 500]
    accumulate over all src tiles. (A-as-stationary pays a 128-col
    LDWEIGHTS per 32-col matmul: ~2x slower. Indexed gather/scatter
    alternatives measured 4-10x slower still.)
  - H'' exchange in TILE-TRANSPOSED layout: each core ships [128, 20*32]
    (tile-major, per-partition contiguous), AllGather -> [1024, 640]; the
    SBUF reload is 8 long-contiguous DMAs. A is tiled over the 160
    (core, local-tile) blocks to match (~2% zero pad vs global tiling;
    the old node-major exchange cost ~30 us in 64-B DMA descriptors).
  - A packed per-partition-contiguous [128, 160*2500] fp8, streamed in
    5-tile (1.6 MB) chunks, triple buffered: DMA ~155 us overlaps the
    ~170 us PE stream.
  - fc1 column-sharded: h1 [32, 2500] -> PE-transpose -> h1T [128, 640];
    640 matmuls lhsT = w1-chunk [128, 128] bf16 (FWL), rhs = h1T column,
    accumulating [128, 1]; w1 bf16 (21 MB/core) streamed in 128-col
    (4.2 MB) chunks (64-col chunks trigger a pathological DMA/LDWEIGHTS
    interaction: 213 us vs 65 us), 2 chunks prefetched during aggregation.
    AllReduce the [128, 1] partials.
  - fc2 column-blocked: 20 matmuls lhsT = w2-block [128, 128] bf16,
    rhs = h3 [128, 1] -> psum [128, 20]; 128-lane epilogue; host
    un-permutes the [128, 20] block layout.
"""
import numpy as np
import ml_dtypes

N = 20000
IN_FEAT = 128
CF = 32            # conv out feats
FC1 = 128
NC_ = 8            # cores
NS = N // NC_      # 2500 nodes per core
LT = 20            # local src tiles per core (last partial: 68 rows)
TT = NC_ * LT      # 160 total src tiles
JW = 5             # dst chunks of 500
JC = NS // JW      # 500 dst cols per chunk
CK = 5             # A src tiles per DMA chunk (1.6 MB)
NCH = TT // CK     # 32 A chunks
GK = 128           # fc1 g-columns per DMA chunk (4.2 MB)
NG = 640           # fc1 contraction columns (20 blocks * 32 feats)
W1BUFS = 3         # fc1 weight chunk buffers

_BF16 = ml_dtypes.bfloat16
_F8 = ml_dtypes.float8_e4m3


def _host_prep(x, edge_index, w_conv, b_conv, w_fc1, b_fc1, w_fc2, b_fc2):
    src = edge_index[0].astype(np.int64)
    dst = edge_index[1].astype(np.int64)
    deg = np.bincount(dst, minlength=N).astype(np.float32) + 1.0
    dinv = (1.0 / np.sqrt(deg)).astype(np.float32)

    x = np.asarray(x, np.float32)
    w_conv = np.asarray(w_conv, np.float32)
    b_conv = np.asarray(b_conv, np.float32)
    w_fc1 = np.asarray(w_fc1, np.float32)
    b_fc1 = np.asarray(b_fc1, np.float32)
    w_fc2 = np.asarray(w_fc2, np.float32)
    b_fc2 = np.asarray(b_fc2, np.float32)

    lut = np.arange(16).astype(_F8)  # exact small-int -> fp8e4m3
    ident = np.eye(CF, dtype=np.float32)
    wconvb = w_conv.astype(_BF16)
    bconvc = np.ascontiguousarray(b_conv.reshape(CF, 1))
    bfc1c = np.ascontiguousarray(b_fc1.reshape(FC1, 1))
    xs = dinv[:, None] * x  # fold dinv[src] into x rows

    # local-tile mapping for a global src id s: tile 20*(s//2500) +
    # (s%2500)//128, partition (s%2500)%128
    s_tile = 20 * (src // NS) + (src % NS) // 128
    s_part = (src % NS) % 128

    in_maps = []
    for c in range(NC_):
        base = c * NS
        # xt: [128 feat, 2560 nodes] bf16 zero-padded, dinv pre-folded
        xt = np.zeros((IN_FEAT, LT * 128), _BF16)
        xt[:, :NS] = xs[base:base + NS].T.astype(_BF16)
        # dinv dst replicated across 32 feat partitions [32, 2500]
        dvrep = np.ascontiguousarray(
            np.broadcast_to(dinv[base:base + NS][None, :], (CF, NS)))
        # A_pack[p, t*2500 + dl] = count(edges src-tile(t,p) -> dst base+dl)
        cnt = np.zeros((128, TT * NS), np.uint8)
        m = (dst >= base) & (dst < base + NS)
        np.add.at(cnt, (s_part[m], s_tile[m] * NS + (dst[m] - base)), 1)
        v = np.arange(base, base + NS)
        np.add.at(cnt, ((v % NS) % 128, (20 * c + (v % NS) // 128) * NS + (v - base)), 1)
        assert cnt.max() < 16, cnt.max()
        apack = lut[cnt]
        del cnt
        # w1pack[p, g*128 + o] = w_fc1[o, 32*(base+128b+p) + f], g = 32b+f
        w1c = w_fc1[:, base * CF:(base + NS) * CF]          # [128, 80000]
        wpad = np.zeros((FC1, LT * 128, CF), np.float32)    # [o, nl, f]
        wpad[:, :NS] = w1c.reshape(FC1, NS, CF)
        w1p = wpad.reshape(FC1, LT, 128, CF).transpose(2, 1, 3, 0)
        w1p = np.ascontiguousarray(w1p.reshape(128, NG * FC1)).astype(_BF16)
        # w2pack[p, b*128 + q] = w_fc2[base + 128b + q, p]
        w2pad = np.zeros((LT * 128, FC1), np.float32)
        w2pad[:NS] = w_fc2[base:base + NS]
        w2p = w2pad.reshape(LT, 128, FC1).transpose(2, 0, 1)
        w2p = np.ascontiguousarray(w2p.reshape(FC1, LT * 128)).astype(_BF16)
        # bias2[q, b] = b_fc2[base + 128b + q] (zero pad)
        b2 = np.zeros(LT * 128, np.float32)
        b2[:NS] = b_fc2[base:base + NS]
        b2 = np.ascontiguousarray(b2.reshape(LT, 128).T)
        in_maps.append({
            "xt": xt,
            "wconv": wconvb,
            "dvrep": dvrep,
            "bconv": bconvc,
            "ident": ident,
            "apack": apack,
            "w1pack": w1p,
            "bfc1": bfc1c,
            "w2pack": w2p,
            "bias2": b2,
        })
    return in_maps


def _build_bass(timing_reps=None, lite=False):
    # lite: timing-diagnostic build — emit only the first A chunk and first
    # w1 chunk (keeps all other phases intact) to measure misc overhead.
    import concourse.bass as bass
    import concourse.mybir as mybir
    import concourse.tile as tile
    from concourse import bacc

    F32, BF16, F8 = mybir.dt.float32, mybir.dt.bfloat16, mybir.dt.float8e4
    import contextlib
    nc = bacc.Bacc("TRN2", target_bir_lowering=False, debug=False,
                   num_devices=1 if timing_reps else NC_)

    xt = nc.dram_tensor("xt", [IN_FEAT, LT * 128], BF16, kind="ExternalInput")
    wconv = nc.dram_tensor("wconv", [IN_FEAT, CF], BF16, kind="ExternalInput")
    dvrep = nc.dram_tensor("dvrep", [CF, NS], F32, kind="ExternalInput")
    bconv = nc.dram_tensor("bconv", [CF, 1], F32, kind="ExternalInput")
    ident = nc.dram_tensor("ident", [CF, CF], F32, kind="ExternalInput")
    apack = nc.dram_tensor("apack", [128, TT * NS], F8, kind="ExternalInput")
    w1pack = nc.dram_tensor("w1pack", [128, NG * FC1], BF16, kind="ExternalInput")
    bfc1 = nc.dram_tensor("bfc1", [FC1, 1], F32, kind="ExternalInput")
    w2pack = nc.dram_tensor("w2pack", [FC1, LT * 128], BF16, kind="ExternalInput")
    bias2 = nc.dram_tensor("bias2", [128, LT], F32, kind="ExternalInput")
    out = nc.dram_tensor("out", [128, LT], F32, kind="ExternalOutput")

    hq_in = nc.dram_tensor("hq_in", [128, LT * CF], BF16)
    hq_out = nc.dram_tensor("hq_out", [NC_ * 128, LT * CF], BF16,
                            addr_space="Shared")
    p_in = nc.dram_tensor("p_in", [FC1, 1], F32)
    p_out = nc.dram_tensor("p_out", [FC1, 1], F32, addr_space="Shared")

    with tile.TileContext(nc) as tc:
        with tc.tile_pool(name="const", bufs=1) as cp, \
             tc.tile_pool(name="abuf", bufs=3) as apool, \
             tc.tile_pool(name="w1buf", bufs=W1BUFS) as wpool, \
             tc.tile_pool(name="work", bufs=2) as wp, \
             tc.tile_pool(name="psA", bufs=2, space="PSUM") as ppA, \
             tc.tile_pool(name="psB", bufs=1, space="PSUM") as ppB, \
             tc.tile_pool(name="psC", bufs=1, space="PSUM") as ppC:

            xt_sb = cp.tile([IN_FEAT, LT * 128], BF16, tag="xt")
            nc.sync.dma_start(out=xt_sb[:], in_=xt[:])
            wconv_sb = cp.tile([IN_FEAT, CF], BF16, tag="wconv")
            nc.sync.dma_start(out=wconv_sb[:], in_=wconv[:])
            dvrep_sb = cp.tile([CF, NS], F32, tag="dvrep")
            nc.sync.dma_start(out=dvrep_sb[:], in_=dvrep[:])
            bconv_sb = cp.tile([CF, 1], F32, tag="bconv")
            nc.sync.dma_start(out=bconv_sb[:], in_=bconv[:])
            ident_sb = cp.tile([CF, CF], F32, tag="ident")
            nc.sync.dma_start(out=ident_sb[:], in_=ident[:])
            bfc1_sb = cp.tile([FC1, 1], F32, tag="bfc1")
            nc.sync.dma_start(out=bfc1_sb[:], in_=bfc1[:])
            w2_sb = cp.tile([FC1, LT * 128], BF16, tag="w2")
            nc.sync.dma_start(out=w2_sb[:], in_=w2pack[:])
            bias2_sb = cp.tile([128, LT], F32, tag="bias2")
            nc.sync.dma_start(out=bias2_sb[:], in_=bias2[:])

            hp = cp.tile([128, TT * CF], BF16, tag="hp")

            loop_cm = tc.For_i(0, timing_reps, 1) if timing_reps else contextlib.nullcontext()
            loop_cm.__enter__()

            # ---- S1: H''_tileT = (dinv*x) @ w_conv, [128, 20*32] bf16 ----
            # (pad rows of xt are zero -> pad rows of H'' are zero)
            hq_sb = cp.tile([128, LT * CF], BF16, tag="hq")
            for k in range(LT):
                ps = ppA.tile([128, CF], F32, space="PSUM", tag="mm")
                nc.tensor.matmul(out=ps[:], lhsT=xt_sb[:, k * 128:(k + 1) * 128],
                                 rhs=wconv_sb[:], start=True, stop=True)
                nc.vector.tensor_copy(out=hq_sb[:, k * CF:(k + 1) * CF], in_=ps[:])
            nc.sync.dma_start(out=hq_in[:], in_=hq_sb[:])

            # ---- S2: AllGather H'' (tile-transposed blocks) ----
            if timing_reps:
                nc.sync.dma_start(out=hq_out[:128], in_=hq_in[:])
            else:
                nc.gpsimd.collective_compute(
                    "AllGather", mybir.AluOpType.bypass,
                    replica_groups=[list(range(NC_))],
                    ins=[hq_in[:]], outs=[hq_out[:]])

            # ---- prefetch: first 3 A chunks run during the AllGather ----
            def a_chunk_dma(t):
                ab = apool.tile([128, CK * NS], F8, tag="apk", name="apk")
                nc.sync.dma_start(out=ab[:],
                                  in_=apack[:, t * CK * NS:(t + 1) * CK * NS])
                return ab

            def w1_chunk_dma(t):
                wb = wpool.tile([128, GK * FC1], BF16, tag="w1", name="wb")
                nc.sync.dma_start(out=wb[:],
                                  in_=w1pack[:, t * GK * FC1:(t + 1) * GK * FC1])
                return wb

            abufs = {t: a_chunk_dma(t) for t in range(1 if lite else 3)}

            # ---- S3: load gathered H'' into SBUF [128, 160*32] ----
            for c in range(NC_):
                nc.sync.dma_start(out=hp[:, c * LT * CF:(c + 1) * LT * CF],
                                  in_=hq_out[c * 128:(c + 1) * 128, :])

            # ---- S4: aggregation: psum[32, 500] x5 += hp_i.T @ A_chunk ----
            # w1 chunks 0-1 prefetch mid-agg (free buffers -> no queue stall)
            wbufs = {}
            psj = [ppB.tile([CF, JC], F32, space="PSUM", tag=f"agg{j}",
                            name=f"agg{j}")
                   for j in range(JW)]
            for t in range(1 if lite else NCH):
                ab = abufs.pop(t) if t in abufs else a_chunk_dma(t)
                if t in (6, 20):
                    wt = (6, 20).index(t)
                    wbufs[wt] = w1_chunk_dma(wt)
                for kl in range(CK):
                    i = t * CK + kl
                    for j in range(JW):
                        nc.tensor.matmul(
                            out=psj[j][:],
                            lhsT=hp[:, i * CF:(i + 1) * CF],
                            rhs=ab[:, kl * NS + j * JC:kl * NS + (j + 1) * JC],
                            start=(i == 0),
                            stop=(i == (CK - 1 if lite else TT - 1)))

            # ---- S5: h1 = relu(dinv_dst * agg + b_conv), f32 [32, 2500] ----
            h1 = cp.tile([CF, NS], F32, tag="h1")
            for j in range(JW):
                t1 = wp.tile([CF, JC], F32, tag="ep")
                nc.vector.tensor_tensor(out=t1[:], in0=psj[j][:],
                                        in1=dvrep_sb[:, j * JC:(j + 1) * JC],
                                        op=mybir.AluOpType.mult)
                nc.scalar.activation(out=h1[:, j * JC:(j + 1) * JC], in_=t1[:],
                                     func=mybir.ActivationFunctionType.Relu,
                                     bias=bconv_sb[:])

            # ---- S6: h1T [128, 640] bf16 via PE transpose ----
            h1T = cp.tile([128, LT * CF], BF16, tag="h1T")
            for b in range(LT):
                n0 = b * 128
                nw = min(128, NS - n0)
                tp = ppA.tile([128, CF], F32, space="PSUM", tag="mm")
                if nw < 128:
                    nc.vector.memset(tp[:], 0.0)
                nc.tensor.transpose(out=tp[:nw], in_=h1[:, n0:n0 + nw],
                                    identity=ident_sb[:])
                nc.vector.tensor_copy(out=h1T[:, b * CF:(b + 1) * CF], in_=tp[:])

            # ---- S7: fc1 partial: ps[128, 1] += w1chunk.T @ h1T_col ----
            psf = ppC.tile([128, 24], F32, space="PSUM", tag="fc")
            for t in range(1 if lite else NG // GK):
                wb = wbufs.pop(t) if t in wbufs else w1_chunk_dma(t)
                for gl in range(GK):
                    g = t * GK + gl
                    nc.tensor.matmul(out=psf[:, 0:1],
                                     lhsT=wb[:, gl * FC1:(gl + 1) * FC1],
                                     rhs=h1T[:, g:g + 1],
                                     start=(g == 0),
                                     stop=(g == (GK - 1 if lite else NG - 1)))
                # stream chunk t+W1BUFS behind chunk t's matmuls
                if not lite and t + W1BUFS < NG // GK:
                    wbufs[t + W1BUFS] = w1_chunk_dma(t + W1BUFS)
            p_sb = cp.tile([FC1, 1], F32, tag="p_sb")
            nc.vector.tensor_copy(out=p_sb[:], in_=psf[:, 0:1])
            nc.sync.dma_start(out=p_in[:], in_=p_sb[:])

            # ---- S8: AllReduce partials, h3 = relu(sum + b_fc1) bf16 ----
            if timing_reps:
                nc.sync.dma_start(out=p_out[:], in_=p_in[:])
            else:
                nc.gpsimd.collective_compute(
                    "AllReduce", mybir.AluOpType.add,
                    replica_groups=[list(range(NC_))],
                    ins=[p_in[:]], outs=[p_out[:]])
            h3f = cp.tile([FC1, 1], F32, tag="h3f")
            nc.sync.dma_start(out=h3f[:], in_=p_out[:])
            h3 = cp.tile([FC1, 1], BF16, tag="h3")
            nc.scalar.activation(out=h3[:], in_=h3f[:],
                                 func=mybir.ActivationFunctionType.Relu,
                                 bias=bfc1_sb[:])

            # ---- S9: fc2 blocks: psum[128, 20]; epilogue on 128 lanes ----
            for b in range(LT):
                nc.tensor.matmul(out=psf[:, 4 + b:5 + b],
                                 lhsT=w2_sb[:, b * 128:(b + 1) * 128],
                                 rhs=h3[:], start=(b == 0), stop=(b == LT - 1))
            o_sb = cp.tile([128, LT], F32, tag="o_sb")
            nc.vector.tensor_tensor(out=o_sb[:], in0=psf[:, 4:4 + LT],
                                    in1=bias2_sb[:], op=mybir.AluOpType.add)
            nc.scalar.activation(out=o_sb[:], in_=o_sb[:],
                                 func=mybir.ActivationFunctionType.Relu)
            nc.sync.dma_start(out=out[:], in_=o_sb[:])
            loop_cm.__exit__(None, None, None) if timing_reps else None

    nc.finalize()
    return nc


_CACHED = {}


def kernel(**inputs) -> np.ndarray:
    from concourse.bass_utils import run_bass_kernel_spmd

    in_maps = _host_prep(**inputs)
    if "nc" not in _CACHED:
        _CACHED["nc"] = _build_bass()
    nc = _CACHED["nc"]
    res = run_bass_kernel_spmd(nc, in_maps, core_ids=list(range(NC_)))
    # out[p, b] = q-value for node base + 128*b + p
    outs = []
    for c in range(NC_):
        o = np.asarray(res.results[c]["out"])  # [128, 20]
        outs.append(o.T.reshape(-1)[:NS])
    return np.concatenate(outs).reshape(1, N)
